# revision 21
# baseline (speedup 1.0000x reference)
"""Trainium2 Bass kernel for an autoregressive LSTM (warmup scan + decode).

Math (Keras LSTMCell, gate order i,f,g,o in the reference):
    z = x @ Wk + h @ Wr + b
    c = sigmoid(f)*c + sigmoid(i)*tanh(g)
    h = sigmoid(o)*tanh(c)
Warmup over T=256 input steps, then S=64 autoregressive decode steps through
a dense head p = h @ Wd + bd fed back as the next input.

Sharding: pure data-parallel over batch, 1024/8 = 128 examples per core
(128 = SBUF partition count). Weights replicated. No collectives.

Per-core layout: z is computed as [batch=128 part, 4096 gates] with the
batch-transposed activations as the matmul stationary operand (bf16) and the
weights streaming, N=512 per PSUM bank. Gate columns are pre-permuted on the
host into NW=4 1024-wide "waves" [i_q|f_q|o_q|g_q] over unit-quarters; each
wave is a 2-bank PSUM tile (pool bufs=3) whose gate math starts while later
waves are still in the matmul stream. Within a wave the matmuls run k-outer
(x first, then hT chunks 0..7) so the next step's PE work never waits on the
previous step's late hT chunks. h is cast to bf16 and transposed back to
[units, batch] chunk-major layout with ONE merged DMA xbar transpose per wave
(~1.25us fixed cost regardless of size), off the compute engines.
"""

import sys

sys.path.insert(0, "/opt/trn_rl_repo")

import numpy as np

import concourse.bass as bass
import concourse.bacc as bacc
import concourse.mybir as mybir
from concourse.tile import TileContext
from concourse.bass_utils import run_bass_kernel_spmd

F32 = mybir.dt.float32
BF16 = mybir.dt.bfloat16
F8 = mybir.dt.float8e4
NPBF16 = mybir.dt.np(mybir.dt.bfloat16)
NPF8 = mybir.dt.np(F8)
AF = mybir.ActivationFunctionType
DR = mybir.MatmulPerfMode.DoubleRow
TAILF = 16                # trailing warmup steps kept pure bf16 for accuracy

B, T, I, U, S = 1024, 256, 64, 1024, 64
NCORES = 8
BC = B // NCORES          # 128 batch per core
KX = I + 1                # x rows + ones row for folded bias
NU = U // 128             # 8 recurrent k-chunks
XBLK = 4                  # warmup steps per input-stream DMA block


NW = 4                    # waves per step (each covers U/NW units, 4U/NW z-cols)
QW = U // NW              # units per wave
WW = 4 * QW               # z columns per wave
NB = WW // 512            # PSUM banks (512-col matmuls) per wave


def _gate_perm():
    """Column permutation: reference gate order [i|f|g|o] (1024 each) ->
    NW waves of [i_q | f_q | o_q | g_q] (QW each)."""
    i0, f0, g0, o0 = 0, U, 2 * U, 3 * U
    parts = []
    for w in range(NW):
        for g in (i0, f0, o0, g0):
            parts.append(np.arange(QW) + g + w * QW)
    return np.concatenate(parts)


def build_nc(n_warm=T, n_dec=S - 1):
    nc = bacc.Bacc()

    nblk = (n_warm + XBLK - 1) // XBLK
    xTbD = nc.declare_dram_parameter("xTb", [nblk, KX, XBLK * BC], BF16, isOutput=False)
    WkD = nc.declare_dram_parameter("Wk", [KX, 4 * U], BF16, isOutput=False)
    WrD = nc.declare_dram_parameter("Wr", [128, NU, 4 * U], BF16, isOutput=False)
    Wr8D = nc.declare_dram_parameter("Wr8", [128, 2, 2, 4 * U], F8, isOutput=False)
    WdD = nc.declare_dram_parameter("Wd", [128, NU, I], BF16, isOutput=False)
    bdD = nc.declare_dram_parameter("bdc", [I, 1], F32, isOutput=False)
    outD = nc.declare_dram_parameter("out", [n_dec + 1, I, BC], F32, isOutput=True)

    with TileContext(nc) as tc:
        with (
            tc.tile_pool(name="const", bufs=1) as cpool,
            tc.tile_pool(name="xp", bufs=2) as xpool,
            tc.tile_pool(name="state", bufs=2) as hpool,
            tc.tile_pool(name="gates", bufs=2) as gpool,
            tc.tile_pool(name="psum", bufs=3, space="PSUM") as zpool,
        ):
            Wk_sb = cpool.tile([KX, 4 * U], BF16)
            Wr_sb = cpool.tile([128, NU, 4 * U], BF16)
            Wr8_sb = cpool.tile([128, 2, 2, 4 * U], F8)
            Wd_sb = cpool.tile([128, NU, I], BF16)
            bd_sb = cpool.tile([I, 1], F32)
            c_sb = cpool.tile([128, U], F32)
            nc.sync.dma_start(Wk_sb[:], WkD[:])
            nc.sync.dma_start(Wr_sb[:], WrD[:])
            nc.sync.dma_start(Wr8_sb[:], Wr8D[:])
            nc.sync.dma_start(Wd_sb[:], WdD[:])
            nc.sync.dma_start(bd_sb[:], bdD[:])
            nc.gpsimd.memset(c_sb[:], 0.0)

            def emit_step(x_lhsT, hT_prev, x_first, hT8_prev=None, make_fp8=False):
                """One LSTM step; returns (hT [128,U] bf16, hT8 or None).

                When hT8_prev is given, unit chunks 0-3 contract as two fp8
                DoubleRow pairs (2x rows per instruction) instead of four
                bf16 k-groups: 72 -> 56 matmuls per step. Chunks 0-3 are
                produced by waves 0,1 (early) and consumed early next step,
                so the extra fp8 cast never sits on the critical chain."""
                hT_new = hpool.tile([128, U], BF16, tag="hT", name="hT_new")
                hT8_new = (
                    hpool.tile([128, 512], F8, tag="hT8", name="hT8_new")
                    if make_fp8
                    else None
                )
                hT8p3 = (
                    hT8_prev.rearrange("p (a b) -> p a b", a=4)
                    if hT8_prev is not None
                    else None
                )
                for w in range(NW):
                    base = WW * w
                    z = zpool.tile([128, WW], F32, tag="z", name="z")
                    ks = []
                    if x_first:
                        ks.append(("x", x_lhsT))
                    if hT_prev is not None:
                        if hT8_prev is not None:
                            ks.append(("p", 0))
                            ks.append(("p", 1))
                            for u in range(4, NU):
                                ks.append(("h", u))
                        else:
                            for u in range(NU):
                                ks.append(("h", u))
                    if not x_first:
                        ks.append(("x", x_lhsT))
                    # k-outer / n-inner: the first-emitted matmuls depend on
                    # operands ready earliest (x, then low hT chunks), so the
                    # PE can start the next step while the previous step's
                    # late hT chunks are still in flight through the
                    # gate-math chain.
                    for ki, (kind, kv) in enumerate(ks):
                        if kind == "x":
                            lhsT = kv
                        elif kind == "p":
                            lhsT = hT8p3[:, 2 * kv : 2 * kv + 2, :]
                        else:
                            lhsT = hT_prev[:, kv * 128 : (kv + 1) * 128]
                        for n in range(NB):
                            c0 = base + n * 512
                            c1 = base + (n + 1) * 512
                            if kind == "x":
                                rhs = Wk_sb[:, c0:c1]
                            elif kind == "p":
                                rhs = Wr8_sb[:, kv, :, c0:c1]
                            else:
                                rhs = Wr_sb[:, kv, c0:c1]
                            nc.tensor.matmul(
                                z[:, n * 512 : (n + 1) * 512],
                                lhsT,
                                rhs,
                                start=(ki == 0),
                                stop=(ki == len(ks) - 1),
                                perf_mode=DR if kind == "p" else None,
                            )
                    # Gate ACT ops split per gate and ordered f,g,i,o to
                    # shorten the serial c-update chain.
                    sig = gpool.tile([128, 3 * QW], F32, tag="sig", name="sig")
                    tg = gpool.tile([128, QW], F32, tag="tg", name="tg")
                    nc.scalar.activation(sig[:, QW : 2 * QW], z[:, QW : 2 * QW], AF.Sigmoid)
                    nc.scalar.activation(tg[:], z[:, 3 * QW : 4 * QW], AF.Tanh)
                    nc.scalar.activation(sig[:, 0:QW], z[:, 0:QW], AF.Sigmoid)
                    nc.scalar.activation(sig[:, 2 * QW : 3 * QW], z[:, 2 * QW : 3 * QW], AF.Sigmoid)
                    cs = c_sb[:, w * QW : (w + 1) * QW]
                    t1 = gpool.tile([128, QW], F32, tag="t1", name="t1")
                    t2 = gpool.tile([128, QW], F32, tag="t2", name="t2")
                    nc.vector.tensor_mul(t1[:], sig[:, QW : 2 * QW], cs)
                    nc.vector.tensor_mul(t2[:], sig[:, 0:QW], tg[:])
                    nc.vector.tensor_add(cs, t1[:], t2[:])
                    tcc = gpool.tile([128, QW], F32, tag="tcc", name="tcc")
                    nc.scalar.activation(tcc[:], cs, AF.Tanh)
                    hbf = gpool.tile([128, QW], BF16, tag="hbf", name="hbf")
                    nc.vector.tensor_mul(hbf[:], sig[:, 2 * QW : 3 * QW], tcc[:])
                    # One merged xbar transpose per wave: DMAT cost is ~1.25us
                    # nearly independent of size, so [128, QW] -> [128, nch, 128]
                    # in a single instruction beats per-chunk transposes.
                    nch = QW // 128
                    hT3 = hT_new.rearrange("p (a b) -> p a b", a=NU)
                    nc.sync.dma_start_transpose(
                        hT3[:, w * nch : (w + 1) * nch, :], hbf[:]
                    )
                    if make_fp8 and w == 1:
                        nc.vector.tensor_copy(hT8_new[:], hT_new[:, 0:512])
                return hT_new, hT8_new

            def emit_dense(hT_cur, out_idx, feedback):
                zp = zpool.tile([128, WW], F32, tag="z", name="zdense")
                pp = zp[0:I, 0:BC]
                for u in range(NU):
                    nc.tensor.matmul(
                        pp,
                        Wd_sb[:, u, :],
                        hT_cur[:, u * 128 : (u + 1) * 128],
                        start=(u == 0),
                        stop=(u == NU - 1),
                    )
                pf = gpool.tile([I, BC], F32, tag="pf", name="pf")
                nc.scalar.activation(pf[:], pp, AF.Identity, bias=bd_sb[:])
                nc.scalar.dma_start(outD[out_idx], pf[:])
                if not feedback:
                    return None
                pt = gpool.tile([KX, BC], BF16, tag="pT", name="pT")
                nc.gpsimd.memset(pt[I : I + 1, :], 1.0)
                nc.scalar.activation(pt[0:I, :], pp, AF.Identity, bias=bd_sb[:])
                return pt

            hT = None
            # prefetch input blocks one block (XBLK steps) ahead so the
            # stream DMA never sits on the first x-matmul's critical path
            nblk_used = (n_warm + XBLK - 1) // XBLK
            xtiles = {}
            if nblk_used > 0:
                xtiles[0] = xpool.tile([KX, XBLK * BC], BF16, tag="xblk", name="xblk")
                nc.sync.dma_start(xtiles[0][:], xTbD[0])
            hT8 = None
            n8 = max(0, n_warm - TAILF)
            for t in range(n_warm):
                b = t // XBLK
                s = t % XBLK
                hT, hT8 = emit_step(
                    xtiles[b][:, s * BC : (s + 1) * BC],
                    hT,
                    x_first=True,
                    hT8_prev=hT8 if t < n8 else None,
                    make_fp8=(t < n8 - 1),
                )
                if t % XBLK == 0 and b + 1 < nblk_used:
                    # prefetch the next input block; emitted after the step so
                    # it queues behind this step's critical hT transposes
                    xtiles[b + 1] = xpool.tile([KX, XBLK * BC], BF16, tag="xblk", name="xblk")
                    nc.sync.dma_start(xtiles[b + 1][:], xTbD[b + 1])
                xtiles.pop(b - 1, None)
            pt = emit_dense(hT, 0, feedback=(n_dec > 0))
            for d in range(n_dec):
                hT, _ = emit_step(pt[:], hT, x_first=False)
                pt = emit_dense(hT, d + 1, feedback=(d < n_dec - 1))

    nc.finalize()
    return nc


def prep_in_maps(inputs, Wk, Wr, b, Wd, bd, n_warm=T):
    """Host-side sharding + layout. inputs [B, T, I] fp32; returns 8 in_maps."""
    perm = _gate_perm()
    Wk_aug = np.concatenate([np.asarray(Wk, np.float32), np.asarray(b, np.float32)[None, :]], axis=0)
    Wk_p = Wk_aug[:, perm].astype(NPBF16)                      # [65, 4096]
    Wr_p = np.asarray(Wr, np.float32)[:, perm]                 # [1024, 4096]
    Wr_p = Wr_p.reshape(NU, 128, 4 * U).transpose(1, 0, 2)     # [128, NU, 4U] f32
    # fp8 DoubleRow pairs for unit chunks 0-3: [p, pair, slot, cols]
    Wr8_p = Wr_p[:, 0:4].reshape(128, 2, 2, 4 * U).astype(NPF8).copy()
    Wr_p = Wr_p.astype(NPBF16).copy()
    Wd_p = np.asarray(Wd, np.float32).reshape(NU, 128, I).transpose(1, 0, 2).astype(NPBF16).copy()
    bd_c = np.asarray(bd, np.float32).reshape(I, 1).copy()

    x = np.asarray(inputs, np.float32)
    nblk = (n_warm + XBLK - 1) // XBLK
    in_maps = []
    for c in range(NCORES):
        xc = x[c * BC : (c + 1) * BC, :n_warm]                 # [BC, n_warm, I]
        xT = np.transpose(xc, (1, 2, 0))                       # [n_warm, I, BC]
        xTa = np.concatenate([xT, np.ones((n_warm, 1, BC), np.float32)], axis=1)
        if nblk * XBLK != n_warm:
            pad = np.zeros((nblk * XBLK - n_warm, KX, BC), np.float32)
            xTa = np.concatenate([xTa, pad], axis=0)
        xTb = (
            xTa.reshape(nblk, XBLK, KX, BC)
            .transpose(0, 2, 1, 3)
            .reshape(nblk, KX, XBLK * BC)
            .astype(NPBF16)
            .copy()
        )
        in_maps.append(
            {"xTb": xTb, "Wk": Wk_p, "Wr": Wr_p, "Wr8": Wr8_p, "Wd": Wd_p, "bdc": bd_c}
        )
    return in_maps


_NC_CACHE = {}


def _get_nc(n_warm, n_dec):
    key = (n_warm, n_dec)
    if key not in _NC_CACHE:
        _NC_CACHE[key] = build_nc(n_warm, n_dec)
    return _NC_CACHE[key]


def run(inputs, Wk, Wr, b, Wd, bd, n_warm, n_dec, trace=False):
    nc = _get_nc(n_warm, n_dec)
    in_maps = prep_in_maps(inputs, Wk, Wr, b, Wd, bd, n_warm)
    res = run_bass_kernel_spmd(nc, in_maps, list(range(NCORES)), trace=trace)
    outs = [np.asarray(res.results[c]["out"], np.float32) for c in range(NCORES)]
    # out[c]: [n_dec+1, I, BC] -> preds [B, n_dec+1, I]
    preds = np.concatenate([o.transpose(2, 0, 1) for o in outs], axis=0)
    return preds, res


def kernel(inputs, Wk, Wr, b, Wd, bd, output_indices, output_steps):
    n_dec = int(output_steps) - 1
    preds, _ = run(inputs, Wk, Wr, b, Wd, bd, T, n_dec)
    idx = np.asarray(output_indices, np.int64)
    return np.take(preds, idx, axis=-1).astype(np.float32)



# revision 22
# speedup vs baseline: 1.1756x; 1.1756x over previous
"""Trainium2 Bass kernel for an autoregressive LSTM (warmup scan + decode).

Math (Keras LSTMCell, gate order i,f,g,o in the reference):
    z = x @ Wk + h @ Wr + b
    c = sigmoid(f)*c + sigmoid(i)*tanh(g)
    h = sigmoid(o)*tanh(c)
Warmup over T=256 input steps, then S=64 autoregressive decode steps through
a dense head p = h @ Wd + bd fed back as the next input.

Sharding: pure data-parallel over batch, 1024/8 = 128 examples per core
(128 = SBUF partition count). Weights replicated. No collectives.

Per-core layout: z is computed as [batch=128 part, 4096 gates] with the
batch-transposed activations as the matmul stationary operand (bf16) and the
weights streaming, N=512 per PSUM bank. Gate columns are pre-permuted on the
host into NW=4 1024-wide "waves" [i_q|f_q|o_q|g_q] over unit-quarters; each
wave is a 2-bank PSUM tile (pool bufs=3) whose gate math starts while later
waves are still in the matmul stream. Within a wave the matmuls run k-outer
(x first, then hT chunks 0..7) so the next step's PE work never waits on the
previous step's late hT chunks. h is cast to bf16 and transposed back to
[units, batch] chunk-major layout with ONE merged DMA xbar transpose per wave
(~1.25us fixed cost regardless of size), off the compute engines.
"""

import sys

sys.path.insert(0, "/opt/trn_rl_repo")

import numpy as np

import concourse.bass as bass
import concourse.bacc as bacc
import concourse.mybir as mybir
from concourse.tile import TileContext
from concourse.bass_utils import run_bass_kernel_spmd

F32 = mybir.dt.float32
BF16 = mybir.dt.bfloat16
F8 = mybir.dt.float8e4
NPBF16 = mybir.dt.np(mybir.dt.bfloat16)
NPF8 = mybir.dt.np(F8)
AF = mybir.ActivationFunctionType
DR = mybir.MatmulPerfMode.DoubleRow
TAILF = 16                # trailing warmup steps kept pure bf16 for accuracy

B, T, I, U, S = 1024, 256, 64, 1024, 64
NCORES = 8
BC = B // NCORES          # 128 batch per core
KX = I + 1                # x rows + ones row for folded bias
NU = U // 128             # 8 recurrent k-chunks
XBLK = 4                  # warmup steps per input-stream DMA block


NW = 4                    # waves per step (each covers U/NW units, 4U/NW z-cols)
QW = U // NW              # units per wave
WW = 4 * QW               # z columns per wave
NB = WW // 512            # PSUM banks (512-col matmuls) per wave


def _gate_perm():
    """Column permutation: reference gate order [i|f|g|o] (1024 each) ->
    NW waves of [i_q | f_q | o_q | g_q] (QW each)."""
    i0, f0, g0, o0 = 0, U, 2 * U, 3 * U
    parts = []
    for w in range(NW):
        for g in (i0, f0, o0, g0):
            parts.append(np.arange(QW) + g + w * QW)
    return np.concatenate(parts)


def build_nc(n_warm=T, n_dec=S - 1):
    nc = bacc.Bacc()

    nblk = (n_warm + XBLK - 1) // XBLK
    xTbD = nc.declare_dram_parameter("xTb", [nblk, KX, XBLK * BC], BF16, isOutput=False)
    WkD = nc.declare_dram_parameter("Wk", [KX, 4 * U], BF16, isOutput=False)
    WrD = nc.declare_dram_parameter("Wr", [128, NU, 4 * U], BF16, isOutput=False)
    Wr8D = nc.declare_dram_parameter("Wr8", [128, 2, 2, 4 * U], F8, isOutput=False)
    WdD = nc.declare_dram_parameter("Wd", [128, NU, I], BF16, isOutput=False)
    bdD = nc.declare_dram_parameter("bdc", [I, 1], F32, isOutput=False)
    outD = nc.declare_dram_parameter("out", [n_dec + 1, I, BC], F32, isOutput=True)

    with TileContext(nc) as tc:
        with (
            tc.tile_pool(name="const", bufs=1) as cpool,
            tc.tile_pool(name="xp", bufs=2) as xpool,
            tc.tile_pool(name="state", bufs=2) as hpool,
            tc.tile_pool(name="gates", bufs=2) as gpool,
            tc.tile_pool(name="psum", bufs=3, space="PSUM") as zpool,
        ):
            Wk_sb = cpool.tile([KX, 4 * U], BF16)
            Wr_sb = cpool.tile([128, NU, 4 * U], BF16)
            Wr8_sb = cpool.tile([128, 2, 2, 4 * U], F8)
            Wd_sb = cpool.tile([128, NU, I], BF16)
            bd_sb = cpool.tile([I, 1], F32)
            c_sb = cpool.tile([128, U], F32)
            nc.sync.dma_start(Wk_sb[:], WkD[:])
            nc.sync.dma_start(Wr_sb[:], WrD[:])
            nc.sync.dma_start(Wr8_sb[:], Wr8D[:])
            nc.sync.dma_start(Wd_sb[:], WdD[:])
            nc.sync.dma_start(bd_sb[:], bdD[:])
            nc.gpsimd.memset(c_sb[:], 0.0)

            def emit_step(x_lhsT, hT_prev, x_first, hT8_prev=None, make_fp8=False):
                """One LSTM step; returns (hT [128,U] bf16, hT8 or None).

                When hT8_prev is given, unit chunks 0-3 contract as two fp8
                DoubleRow pairs (2x rows per instruction) instead of four
                bf16 k-groups: 72 -> 56 matmuls per step. Chunks 0-3 are
                produced by waves 0,1 (early) and consumed early next step,
                so the extra fp8 cast never sits on the critical chain."""
                hT_new = hpool.tile([128, U], BF16, tag="hT", name="hT_new")
                hT8_new = (
                    hpool.tile([128, 512], F8, tag="hT8", name="hT8_new")
                    if make_fp8
                    else None
                )
                hT8p3 = (
                    hT8_prev.rearrange("p (a b) -> p a b", a=4)
                    if hT8_prev is not None
                    else None
                )
                for w in range(NW):
                    base = WW * w
                    z = zpool.tile([128, WW], F32, tag="z", name="z")
                    ks = []
                    if x_first:
                        ks.append(("x", x_lhsT))
                    if hT_prev is not None:
                        if hT8_prev is not None:
                            ks.append(("p", 0))
                            ks.append(("p", 1))
                            for u in range(4, NU):
                                ks.append(("h", u))
                        else:
                            for u in range(NU):
                                ks.append(("h", u))
                    if not x_first:
                        ks.append(("x", x_lhsT))
                    # k-outer / n-inner: the first-emitted matmuls depend on
                    # operands ready earliest (x, then low hT chunks), so the
                    # PE can start the next step while the previous step's
                    # late hT chunks are still in flight through the
                    # gate-math chain.
                    for ki, (kind, kv) in enumerate(ks):
                        if kind == "x":
                            lhsT = kv
                        elif kind == "p":
                            lhsT = hT8p3[:, 2 * kv : 2 * kv + 2, :]
                        else:
                            lhsT = hT_prev[:, kv * 128 : (kv + 1) * 128]
                        for n in range(NB):
                            c0 = base + n * 512
                            c1 = base + (n + 1) * 512
                            if kind == "x":
                                rhs = Wk_sb[:, c0:c1]
                            elif kind == "p":
                                rhs = Wr8_sb[:, kv, :, c0:c1]
                            else:
                                rhs = Wr_sb[:, kv, c0:c1]
                            nc.tensor.matmul(
                                z[:, n * 512 : (n + 1) * 512],
                                lhsT,
                                rhs,
                                start=(ki == 0),
                                stop=(ki == len(ks) - 1),
                                perf_mode=DR if kind == "p" else None,
                            )
                    # Gate ACT ops split per gate and ordered f,g,i,o to
                    # shorten the serial c-update chain.
                    sig = gpool.tile([128, 3 * QW], F32, tag="sig", name="sig")
                    tg = gpool.tile([128, QW], F32, tag="tg", name="tg")
                    nc.scalar.activation(sig[:, QW : 2 * QW], z[:, QW : 2 * QW], AF.Sigmoid)
                    nc.scalar.activation(tg[:], z[:, 3 * QW : 4 * QW], AF.Tanh)
                    nc.scalar.activation(sig[:, 0:QW], z[:, 0:QW], AF.Sigmoid)
                    nc.scalar.activation(sig[:, 2 * QW : 3 * QW], z[:, 2 * QW : 3 * QW], AF.Sigmoid)
                    cs = c_sb[:, w * QW : (w + 1) * QW]
                    t1 = gpool.tile([128, QW], F32, tag="t1", name="t1")
                    t2 = gpool.tile([128, QW], F32, tag="t2", name="t2")
                    nc.vector.tensor_mul(t1[:], sig[:, QW : 2 * QW], cs)
                    nc.vector.tensor_mul(t2[:], sig[:, 0:QW], tg[:])
                    nc.vector.tensor_add(cs, t1[:], t2[:])
                    tcc = gpool.tile([128, QW], F32, tag="tcc", name="tcc")
                    nc.scalar.activation(tcc[:], cs, AF.Tanh)
                    hbf = gpool.tile([128, QW], BF16, tag="hbf", name="hbf")
                    nc.vector.tensor_mul(hbf[:], sig[:, 2 * QW : 3 * QW], tcc[:])
                    # One merged xbar transpose per wave: DMAT cost is ~1.25us
                    # nearly independent of size, so [128, QW] -> [128, nch, 128]
                    # in a single instruction beats per-chunk transposes.
                    nch = QW // 128
                    hT3 = hT_new.rearrange("p (a b) -> p a b", a=NU)
                    nc.sync.dma_start_transpose(
                        hT3[:, w * nch : (w + 1) * nch, :], hbf[:]
                    )
                    if make_fp8 and w == 1:
                        nc.vector.tensor_copy(hT8_new[:], hT_new[:, 0:512])
                return hT_new, hT8_new

            def emit_dense(hT_cur, out_idx, feedback):
                zp = zpool.tile([128, WW], F32, tag="z", name="zdense")
                pp = zp[0:I, 0:BC]
                for u in range(NU):
                    nc.tensor.matmul(
                        pp,
                        Wd_sb[:, u, :],
                        hT_cur[:, u * 128 : (u + 1) * 128],
                        start=(u == 0),
                        stop=(u == NU - 1),
                    )
                pf = gpool.tile([I, BC], F32, tag="pf", name="pf")
                nc.scalar.activation(pf[:], pp, AF.Identity, bias=bd_sb[:])
                nc.scalar.dma_start(outD[out_idx], pf[:])
                if not feedback:
                    return None
                pt = gpool.tile([KX, BC], BF16, tag="pT", name="pT")
                nc.gpsimd.memset(pt[I : I + 1, :], 1.0)
                nc.scalar.activation(pt[0:I, :], pp, AF.Identity, bias=bd_sb[:])
                return pt

            hT = None
            # prefetch input blocks one block (XBLK steps) ahead so the
            # stream DMA never sits on the first x-matmul's critical path
            nblk_used = (n_warm + XBLK - 1) // XBLK
            xtiles = {}
            if nblk_used > 0:
                xtiles[0] = xpool.tile([KX, XBLK * BC], BF16, tag="xblk", name="xblk")
                nc.sync.dma_start(xtiles[0][:], xTbD[0])
            hT8 = None
            n8 = max(0, n_warm - TAILF)
            for t in range(n_warm):
                b = t // XBLK
                s = t % XBLK
                hT, hT8 = emit_step(
                    xtiles[b][:, s * BC : (s + 1) * BC],
                    hT,
                    x_first=True,
                    hT8_prev=hT8 if t < n8 else None,
                    make_fp8=(t < n8 - 1),
                )
                if t % XBLK == 0 and b + 1 < nblk_used:
                    # prefetch the next input block; emitted after the step so
                    # it queues behind this step's critical hT transposes
                    xtiles[b + 1] = xpool.tile([KX, XBLK * BC], BF16, tag="xblk", name="xblk")
                    nc.sync.dma_start(xtiles[b + 1][:], xTbD[b + 1])
                xtiles.pop(b - 1, None)
            pt = emit_dense(hT, 0, feedback=(n_dec > 0))
            hT8 = None
            for d in range(n_dec):
                hT, hT8 = emit_step(
                    pt[:],
                    hT,
                    x_first=False,
                    hT8_prev=hT8,
                    make_fp8=(d < n_dec - 1),
                )
                pt = emit_dense(hT, d + 1, feedback=(d < n_dec - 1))

    nc.finalize()
    return nc


def prep_in_maps(inputs, Wk, Wr, b, Wd, bd, n_warm=T):
    """Host-side sharding + layout. inputs [B, T, I] fp32; returns 8 in_maps."""
    perm = _gate_perm()
    Wk_aug = np.concatenate([np.asarray(Wk, np.float32), np.asarray(b, np.float32)[None, :]], axis=0)
    Wk_p = Wk_aug[:, perm].astype(NPBF16)                      # [65, 4096]
    Wr_p = np.asarray(Wr, np.float32)[:, perm]                 # [1024, 4096]
    Wr_p = Wr_p.reshape(NU, 128, 4 * U).transpose(1, 0, 2)     # [128, NU, 4U] f32
    # fp8 DoubleRow pairs for unit chunks 0-3: [p, pair, slot, cols]
    Wr8_p = Wr_p[:, 0:4].reshape(128, 2, 2, 4 * U).astype(NPF8).copy()
    Wr_p = Wr_p.astype(NPBF16).copy()
    Wd_p = np.asarray(Wd, np.float32).reshape(NU, 128, I).transpose(1, 0, 2).astype(NPBF16).copy()
    bd_c = np.asarray(bd, np.float32).reshape(I, 1).copy()

    x = np.asarray(inputs, np.float32)
    nblk = (n_warm + XBLK - 1) // XBLK
    in_maps = []
    for c in range(NCORES):
        xc = x[c * BC : (c + 1) * BC, :n_warm]                 # [BC, n_warm, I]
        xT = np.transpose(xc, (1, 2, 0))                       # [n_warm, I, BC]
        xTa = np.concatenate([xT, np.ones((n_warm, 1, BC), np.float32)], axis=1)
        if nblk * XBLK != n_warm:
            pad = np.zeros((nblk * XBLK - n_warm, KX, BC), np.float32)
            xTa = np.concatenate([xTa, pad], axis=0)
        xTb = (
            xTa.reshape(nblk, XBLK, KX, BC)
            .transpose(0, 2, 1, 3)
            .reshape(nblk, KX, XBLK * BC)
            .astype(NPBF16)
            .copy()
        )
        in_maps.append(
            {"xTb": xTb, "Wk": Wk_p, "Wr": Wr_p, "Wr8": Wr8_p, "Wd": Wd_p, "bdc": bd_c}
        )
    return in_maps


_NC_CACHE = {}


def _get_nc(n_warm, n_dec):
    key = (n_warm, n_dec)
    if key not in _NC_CACHE:
        _NC_CACHE[key] = build_nc(n_warm, n_dec)
    return _NC_CACHE[key]


def run(inputs, Wk, Wr, b, Wd, bd, n_warm, n_dec, trace=False):
    nc = _get_nc(n_warm, n_dec)
    in_maps = prep_in_maps(inputs, Wk, Wr, b, Wd, bd, n_warm)
    res = run_bass_kernel_spmd(nc, in_maps, list(range(NCORES)), trace=trace)
    outs = [np.asarray(res.results[c]["out"], np.float32) for c in range(NCORES)]
    # out[c]: [n_dec+1, I, BC] -> preds [B, n_dec+1, I]
    preds = np.concatenate([o.transpose(2, 0, 1) for o in outs], axis=0)
    return preds, res


def kernel(inputs, Wk, Wr, b, Wd, bd, output_indices, output_steps):
    n_dec = int(output_steps) - 1
    preds, _ = run(inputs, Wk, Wr, b, Wd, bd, T, n_dec)
    idx = np.asarray(output_indices, np.int64)
    return np.take(preds, idx, axis=-1).astype(np.float32)

